# revision 18
# baseline (speedup 1.0000x reference)
"""Trainium2 Bass kernel for nn_Attention (B=2, S=2048, D=1024, H=16, causal).

Sharding: head-parallel across 8 NeuronCores — 2 heads per core. Each core:
  1. computes qT/kT/vT for its 2 heads from the full xT (QKV projection,
     transposed layout [128 = 2*hd, S]),
  2. runs causal attention per head with scores in transposed orientation
     (sT[sj, si]) so the PV matmul needs no P transpose; the softmax
     denominator comes free as an extra ones-column in the V operand,
  3. multiplies by its 128-row slice of W_proj producing a partial output
     yT_c [B, D, S].
Host sums the 8 partials, adds b_proj, and transposes back to [B, S, D].

v3:
  - all matmul operands bf16 (1 cycle/row, fast weight load), fp32 PSUM.
  - causal mask folded into the scores matmul: a constant strict-upper
    [128,128] matrix of -240 is the first matmul of the scores PSUM
    accumulation group (exp(0.125*-240)=exp(-30)->0), so ee needs no
    separate mask multiply and the exp->PV chain has no vector/gpsimd hop.
  - lag-2 software pipeline in the j loop: PV(j-2) is emitted after
    scores(j), so the PE never stalls waiting for the scalar Exp.
  - softmax denominator reciprocal via the fast custom-DVE approx op,
    broadcast across partitions with a rank-1 PE matmul.
  - x loaded as full [128, S] rows (16 DMAs, both batches upfront).
"""
import sys

sys.path.insert(0, "/opt/trn_rl_repo")

import numpy as np
import ml_dtypes
import concourse.bacc as bacc
import concourse.mybir as mybir
import concourse.tile as tile
from concourse.bass_utils import run_bass_kernel_spmd

dt = mybir.dt
BF16 = dt.bfloat16
AF = mybir.ActivationFunctionType

B, S, D, H = 2, 2048, 1024, 16
HD = D // H            # 64
NCORE = 8
HPC = H // NCORE       # 2 heads per core

_CACHE = {}


def build_nc():
    nc = bacc.Bacc("TRN2", target_bir_lowering=False, debug=False)

    xT_d = nc.dram_tensor("xT", [B, D, S], BF16, kind="ExternalInput")
    wq_d = nc.dram_tensor("wq", [128, 8, 128], BF16, kind="ExternalInput")
    wk_d = nc.dram_tensor("wk", [128, 8, 128], BF16, kind="ExternalInput")
    wv_d = nc.dram_tensor("wv", [128, 8, 128], BF16, kind="ExternalInput")
    bq_d = nc.dram_tensor("bq", [128, 1], dt.float32, kind="ExternalInput")
    bk_d = nc.dram_tensor("bk", [128, 1], dt.float32, kind="ExternalInput")
    bv_d = nc.dram_tensor("bv", [128, 1], dt.float32, kind="ExternalInput")
    wp_d = nc.dram_tensor("wp", [128, D], BF16, kind="ExternalInput")
    msk_d = nc.dram_tensor("msku", [128, 128], BF16, kind="ExternalInput")
    id_d = nc.dram_tensor("ident", [128, 128], BF16, kind="ExternalInput")
    ones_d = nc.dram_tensor("ones", [128, 64], BF16, kind="ExternalInput")
    zer_d = nc.dram_tensor("zer", [64, S], BF16, kind="ExternalInput")
    yT_d = nc.dram_tensor("yT", [B, D, S], BF16, kind="ExternalOutput")

    with tile.TileContext(nc) as tc:
        with (
            tc.tile_pool(name="consts", bufs=1) as consts,
            tc.tile_pool(name="xpool", bufs=16) as xpool,
            tc.tile_pool(name="vpool", bufs=1) as vpool,
            tc.tile_pool(name="qkv", bufs=2) as qkvp,
            tc.tile_pool(name="epool", bufs=6) as epool,
            tc.tile_pool(name="ypool", bufs=6) as ypool,
            tc.tile_pool(name="rpool", bufs=3) as rpool,
            tc.tile_pool(name="ps_mm2", bufs=2, space="PSUM") as ps_mm2,
            tc.tile_pool(name="ps_a", bufs=2, space="PSUM") as ps_a_pool,
            tc.tile_pool(name="ps_aux", bufs=2, space="PSUM") as ps_aux,
        ):
            # ---- constants / weights (once, one DMA each) ----
            wvr = consts.tile([128, 8, 128], BF16, tag="wv")
            wkr = consts.tile([128, 8, 128], BF16, tag="wk")
            wqr = consts.tile([128, 8, 128], BF16, tag="wq")
            nc.gpsimd.dma_start(wvr[:], wv_d.ap()[:])
            nc.sync.dma_start(wkr[:], wk_d.ap()[:])
            nc.scalar.dma_start(wqr[:], wq_d.ap()[:])
            wpr = consts.tile([128, D], BF16, tag="wp")
            nc.gpsimd.dma_start(wpr[:], wp_d.ap()[:])
            bq_sb = consts.tile([128, 1], dt.float32, tag="bq")
            bk_sb = consts.tile([128, 1], dt.float32, tag="bk")
            bv_sb = consts.tile([128, 1], dt.float32, tag="bv")
            nc.gpsimd.dma_start(bq_sb[:], bq_d.ap()[:])
            nc.gpsimd.dma_start(bk_sb[:], bk_d.ap()[:])
            nc.gpsimd.dma_start(bv_sb[:], bv_d.ap()[:])
            msku = consts.tile([128, 128], BF16, tag="msku")
            nc.gpsimd.dma_start(msku[:], msk_d.ap()[:])
            ident = consts.tile([128, 128], BF16, tag="ident")
            nc.gpsimd.dma_start(ident[:], id_d.ap()[:])
            ones_r = consts.tile([128, 64], BF16, tag="ones")
            nc.gpsimd.dma_start(ones_r[:], ones_d.ap()[:, :])

            # ---- x loads: full rows, both batches, upfront ----
            xd = [
                [
                    xpool.tile([128, S], BF16, tag="x", name=f"x_{b}_{d}")
                    for d in range(8)
                ]
                for b in range(B)
            ]
            for b in range(B):
                for d in range(8):
                    eng = nc.sync if d % 2 == 0 else nc.scalar
                    eng.dma_start(
                        xd[b][d][:],
                        xT_d.ap()[b, 128 * d:128 * (d + 1), :],
                    )

            # pending proj work items: (b, blk, dtile, aT) emitted one at a
            # time, spread through the next block's j loop so the PSUM ring
            # and the DVE y-copies never gate the PE.
            proj_q = []

            def emit_proj_one(copy_scalar=False):
                if not proj_q:
                    return
                b, blk, dtile, aT = proj_q.pop(0)
                si0 = 512 * blk
                ps = ps_aux.tile([128, 512], dt.float32, tag="aux",
                                 name=f"psp_{b}_{blk}_{dtile}")
                nc.tensor.matmul(
                    ps[:],
                    wpr[:, 128 * dtile:128 * (dtile + 1)],
                    aT[:, si0:si0 + 512],
                    start=True,
                    stop=True,
                )
                y_sb = ypool.tile([128, 512], BF16, tag="y",
                                  name=f"y_{b}_{blk}_{dtile}")
                with nc.allow_low_precision(reason="bf16 partial out"):
                    if copy_scalar:
                        nc.scalar.activation(y_sb[:], ps[:], AF.Copy)
                    else:
                        nc.vector.tensor_copy(y_sb[:], ps[:])
                dma_eng = nc.sync if dtile % 2 == 0 else nc.gpsimd
                dma_eng.dma_start(
                    yT_d.ap()[
                        b, 128 * dtile:128 * (dtile + 1), si0:si0 + 512,
                    ],
                    y_sb[:],
                )

            def queue_proj(b, blk, aT):
                for dtile in range(8):
                    proj_q.append((b, blk, dtile, aT))

            for b in range(B):
                qTr = qkvp.tile([128, S], BF16, tag="qT", name=f"qT_{b}")
                kp0 = qkvp.tile([128, S], BF16, tag="kp0", name=f"kp0_{b}")
                kp1 = qkvp.tile([128, S], BF16, tag="kp1", name=f"kp1_{b}")
                nc.gpsimd.dma_start(kp0[64:128, :], zer_d.ap()[:])
                nc.gpsimd.dma_start(kp1[0:64, :], zer_d.ap()[:])
                vT = vpool.tile([128, S], BF16, tag="vT", name=f"vT_{b}")

                for bp in range(2):
                    for (w_r, bias, kind) in (
                        (wvr, bv_sb, "v"),
                        (wkr, bk_sb, "k"),
                        (wqr, bq_sb, "q"),
                    ):
                        pp = ps_mm2.tile([128, 2, 512], dt.float32, tag="mm2",
                                         name=f"qkv_{b}_{kind}_{bp}")
                        for d in range(8):
                            for t in range(2):
                                blk = 2 * bp + t
                                nc.tensor.matmul(
                                    pp[:, t, :],
                                    w_r[:, d, :],
                                    xd[b][d][:, 512 * blk:512 * (blk + 1)],
                                    start=(d == 0),
                                    stop=(d == 7),
                                )
                        cols = slice(1024 * bp, 1024 * (bp + 1))
                        src_ap = pp[:].rearrange("p t f -> p (t f)")
                        with nc.allow_low_precision(reason="bf16 qkv bias"):
                            if kind == "q":
                                nc.vector.tensor_scalar_add(
                                    qTr[:, cols], src_ap, bias[:, 0:1])
                            elif kind == "v":
                                nc.vector.tensor_scalar_add(
                                    vT[:, cols], src_ap, bias[:, 0:1])
                            else:
                                nc.vector.tensor_scalar_add(
                                    kp0[0:64, cols], src_ap[0:64, :],
                                    bias[0:64, 0:1])
                                nc.vector.tensor_scalar_add(
                                    kp1[64:128, cols], src_ap[64:128, :],
                                    bias[64:128, 0:1])

                # vhat: v natural per sj tile + ones column, bf16.
                vhat = qkvp.tile([128, 16, 130], BF16, tag="vhat",
                                 name=f"vhat_{b}")
                nc.gpsimd.dma_start(vhat[:, :, 64], ones_d.ap()[:, 0:16])
                nc.gpsimd.dma_start(vhat[:, :, 129], ones_d.ap()[:, 16:32])

                def emit_vhat(j):
                    pst = ps_aux.tile([128, 128], BF16, tag="aux",
                                      name=f"tr_{b}_{j}")
                    nc.tensor.transpose(
                        pst[:], vT[:, 128 * j:128 * (j + 1)], ident[:]
                    )
                    with nc.allow_low_precision(reason="bf16 vhat"):
                        nc.vector.tensor_copy(vhat[:, j, 0:64], pst[:, 0:64])
                        nc.vector.tensor_copy(vhat[:, j, 65:129],
                                              pst[:, 64:128])

                aT = qkvp.tile([128, S], BF16, tag="aT", name=f"aT_{b}")

                # ---- causal attention, lag-2 pipelined over j ----
                for blk in range(4):
                    si0 = 512 * blk
                    jlast = 4 * blk + 3
                    for j in range(4 * blk, 4 * blk + 4):
                        emit_vhat(j)
                    psa = [
                        ps_a_pool.tile([65, 512], dt.float32, tag="acc",
                                       name=f"psa_{b}_{blk}_{hl}")
                        for hl in range(HPC)
                    ]
                    ees = {}

                    def emit_scores(j):
                        off = max(0, 128 * (j - 4 * blk))
                        w = 512 - off
                        pp = ps_mm2.tile([128, 2, 512], dt.float32, tag="mm2",
                                         name=f"pp_{b}_{blk}_{j}")
                        diag = j >= 4 * blk
                        for hl, kp in ((0, kp0), (1, kp1)):
                            if diag:
                                # causal mask: -240 strict-upper, first in
                                # the accumulation group
                                nc.tensor.matmul(
                                    pp[:, hl, 0:128],
                                    msku[:],
                                    ident[:],
                                    start=True,
                                    stop=False,
                                )
                            nc.tensor.matmul(
                                pp[:, hl, 0:w],
                                kp[:, 128 * j:128 * (j + 1)],
                                qTr[:, si0 + off:si0 + 512],
                                start=not diag,
                                stop=True,
                            )
                        ee = epool.tile([128, 2, 512], BF16, tag="eT",
                                        name=f"ee_{b}_{blk}_{j}")
                        nc.scalar.activation(
                            ee[:, :, 0:w], pp[:, :, 0:w], AF.Exp, scale=0.125
                        )
                        ees[j] = (ee, off, w)

                    def emit_pv(j):
                        ee, off, w = ees.pop(j)
                        for hl in range(HPC):
                            nc.tensor.matmul(
                                psa[hl][:, off:512],
                                vhat[:, j, 65 * hl:65 * hl + 65],
                                ee[:, hl, 0:w],
                                start=(j == 0),
                                stop=(j == jlast),
                            )

                    LAG = 2
                    nproj = 3 if jlast + 1 < 8 else 1
                    for j in range(jlast + 1):
                        emit_scores(j)
                        if j >= LAG:
                            emit_pv(j - LAG)
                        if j >= 1:
                            for _ in range(nproj):
                                emit_proj_one()
                    for j in range(max(0, jlast + 1 - LAG), jlast + 1):
                        emit_pv(j)

                    # start the denominator/reciprocal chain on scalar+DVE
                    # immediately; the PE does proj(blk-1) meanwhile, so the
                    # psb broadcast matmul never waits.
                    recs = []
                    for hl in range(HPC):
                        den = rpool.tile([1, 512], dt.float32, tag="den",
                                         name=f"den_{b}_{blk}_{hl}")
                        nc.scalar.activation(den[:], psa[hl][64:65, :],
                                             AF.Copy)
                        rec_f = rpool.tile([1, 512], dt.float32, tag="rec_f",
                                           name=f"recf_{b}_{blk}_{hl}")
                        nc.vector.reciprocal_approx_fast(rec_f[:], den[:])
                        rec = rpool.tile([1, 512], BF16, tag="rec",
                                         name=f"rec_{b}_{blk}_{hl}")
                        with nc.allow_low_precision(reason="bf16 recip"):
                            nc.vector.tensor_copy(rec[:], rec_f[:])
                        recs.append(rec)
                    while proj_q:
                        emit_proj_one()
                    for hl in range(HPC):
                        p0 = 64 * hl
                        psb = ps_aux.tile([64, 512], dt.float32, tag="aux",
                                          name=f"psb_{b}_{blk}_{hl}")
                        nc.tensor.matmul(
                            psb[:], ones_r[0:1, 0:64], recs[hl][:],
                            start=True, stop=True
                        )
                        a_sb = rpool.tile([64, 512], BF16, tag="a_sb",
                                          name=f"asb_{b}_{blk}_{hl}")
                        with nc.allow_low_precision(reason="bf16 attn norm"):
                            nc.vector.tensor_copy(a_sb[:], psa[hl][0:64, :])
                            nc.vector.tensor_mul(
                                aT[p0:p0 + 64, si0:si0 + 512],
                                a_sb[:],
                                psb[:],
                            )
                    queue_proj(b, blk, aT)
            k = 0
            while proj_q:
                emit_proj_one(copy_scalar=(k % 2 == 1))
                k += 1
    nc.compile()
    return nc


def _get_nc():
    if "nc" not in _CACHE:
        _CACHE["nc"] = build_nc()
    return _CACHE["nc"]


def prep_w(w):
    # [1024, 128] -> [128(p), 8(d), 128(m)] so the SBUF load is contiguous
    return np.ascontiguousarray(
        w.reshape(8, 128, 128).transpose(1, 0, 2)
    ).astype(ml_dtypes.bfloat16)


def make_in_maps(x, W_attn, b_attn, W_proj):
    x = np.ascontiguousarray(x, dtype=np.float32)
    xT = np.ascontiguousarray(x.transpose(0, 2, 1)).astype(ml_dtypes.bfloat16)

    p = np.arange(128)
    # strict upper-triangular -240: out[sj, si] += msku[si, sj] via
    # matmul(lhsT=msku, rhs=ident) -> -240 where sj > si
    msku = np.where(p[None, :] > p[:, None], -240.0, 0.0)
    msku = msku.astype(ml_dtypes.bfloat16)
    ident = np.eye(128, dtype=np.float32).astype(ml_dtypes.bfloat16)
    ones = np.ones((128, 64), ml_dtypes.bfloat16)

    in_maps = []
    for c in range(NCORE):
        col0 = HD * HPC * c
        in_maps.append({
            "xT": xT,
            "wq": prep_w(W_attn[:, col0:col0 + 128]),
            "wk": prep_w(W_attn[:, D + col0:D + col0 + 128]),
            "wv": prep_w(W_attn[:, 2 * D + col0:2 * D + col0 + 128]),
            "bq": np.ascontiguousarray(b_attn[col0:col0 + 128].reshape(128, 1)).astype(np.float32),
            "bk": np.ascontiguousarray(b_attn[D + col0:D + col0 + 128].reshape(128, 1)).astype(np.float32),
            "bv": np.ascontiguousarray(b_attn[2 * D + col0:2 * D + col0 + 128].reshape(128, 1)).astype(np.float32),
            "wp": np.ascontiguousarray(W_proj[128 * c:128 * (c + 1), :]).astype(ml_dtypes.bfloat16),
            "msku": msku,
            "zer": np.zeros((64, S), ml_dtypes.bfloat16),
            "ident": ident,
            "ones": ones,
        })
    return in_maps


def gather(results, b_proj):
    acc = np.zeros((B, D, S), np.float32)
    for r in results:
        acc += np.asarray(r["yT"], dtype=np.float32)
    out = acc.transpose(0, 2, 1) + np.asarray(b_proj, np.float32)[None, None, :]
    return np.ascontiguousarray(out.astype(np.float32))


def kernel(x, W_attn, b_attn, W_proj, b_proj, _trace=False, _trace_kwargs=None):
    nc = _get_nc()
    in_maps = make_in_maps(np.asarray(x), np.asarray(W_attn),
                           np.asarray(b_attn), np.asarray(W_proj))
    res = run_bass_kernel_spmd(
        nc, in_maps, list(range(NCORE)), trace=_trace, **(_trace_kwargs or {})
    )
    out = gather(res.results, np.asarray(b_proj))
    if _trace:
        kernel.last_result = res
    return out


# revision 19
# speedup vs baseline: 1.0280x; 1.0280x over previous
"""Trainium2 Bass kernel for nn_Attention (B=2, S=2048, D=1024, H=16, causal).

Sharding: head-parallel across 8 NeuronCores — 2 heads per core. Each core:
  1. computes qT/kT/vT for its 2 heads from the full xT (QKV projection,
     transposed layout [128 = 2*hd, S]),
  2. runs causal attention per head with scores in transposed orientation
     (sT[sj, si]) so the PV matmul needs no P transpose; the softmax
     denominator comes free as an extra ones-column in the V operand,
  3. multiplies by its 128-row slice of W_proj producing a partial output
     yT_c [B, D, S].
Host sums the 8 partials, adds b_proj, and transposes back to [B, S, D].

v3:
  - all matmul operands bf16 (1 cycle/row, fast weight load), fp32 PSUM.
  - causal mask folded into the scores matmul: a constant strict-upper
    [128,128] matrix of -240 is the first matmul of the scores PSUM
    accumulation group (exp(0.125*-240)=exp(-30)->0), so ee needs no
    separate mask multiply and the exp->PV chain has no vector/gpsimd hop.
  - lag-2 software pipeline in the j loop: PV(j-2) is emitted after
    scores(j), so the PE never stalls waiting for the scalar Exp.
  - softmax denominator reciprocal via the fast custom-DVE approx op,
    broadcast across partitions with a rank-1 PE matmul.
  - x loaded as full [128, S] rows (16 DMAs, both batches upfront).
"""
import sys

sys.path.insert(0, "/opt/trn_rl_repo")

import numpy as np
import ml_dtypes
import concourse.bacc as bacc
import concourse.mybir as mybir
import concourse.tile as tile
from concourse.bass_utils import run_bass_kernel_spmd

dt = mybir.dt
BF16 = dt.bfloat16
AF = mybir.ActivationFunctionType

B, S, D, H = 2, 2048, 1024, 16
HD = D // H            # 64
NCORE = 8
HPC = H // NCORE       # 2 heads per core

_CACHE = {}


def build_nc():
    nc = bacc.Bacc("TRN2", target_bir_lowering=False, debug=False)

    xT_d = nc.dram_tensor("xT", [B, D, S], BF16, kind="ExternalInput")
    wq_d = nc.dram_tensor("wq", [128, 8, 128], BF16, kind="ExternalInput")
    wk_d = nc.dram_tensor("wk", [128, 8, 128], BF16, kind="ExternalInput")
    wv_d = nc.dram_tensor("wv", [128, 8, 128], BF16, kind="ExternalInput")
    bq_d = nc.dram_tensor("bq", [128, 1], dt.float32, kind="ExternalInput")
    bk_d = nc.dram_tensor("bk", [128, 1], dt.float32, kind="ExternalInput")
    bv_d = nc.dram_tensor("bv", [128, 1], dt.float32, kind="ExternalInput")
    wp_d = nc.dram_tensor("wp", [128, D], BF16, kind="ExternalInput")
    msk_d = nc.dram_tensor("msku", [128, 128], BF16, kind="ExternalInput")
    id_d = nc.dram_tensor("ident", [128, 128], BF16, kind="ExternalInput")
    ones_d = nc.dram_tensor("ones", [128, 64], BF16, kind="ExternalInput")
    zer_d = nc.dram_tensor("zer", [64, S], BF16, kind="ExternalInput")
    yT_d = nc.dram_tensor("yT", [B, D, S], BF16, kind="ExternalOutput")

    with tile.TileContext(nc) as tc:
        with (
            tc.tile_pool(name="consts", bufs=1) as consts,
            tc.tile_pool(name="xpool", bufs=16) as xpool,
            tc.tile_pool(name="vpool", bufs=1) as vpool,
            tc.tile_pool(name="qkv", bufs=2) as qkvp,
            tc.tile_pool(name="epool", bufs=6) as epool,
            tc.tile_pool(name="ypool", bufs=6) as ypool,
            tc.tile_pool(name="rpool", bufs=3) as rpool,
            tc.tile_pool(name="ps_mm2", bufs=2, space="PSUM") as ps_mm2,
            tc.tile_pool(name="ps_a", bufs=2, space="PSUM") as ps_a_pool,
            tc.tile_pool(name="ps_aux", bufs=2, space="PSUM") as ps_aux,
        ):
            # ---- constants / weights (once, one DMA each) ----
            wvr = consts.tile([128, 8, 128], BF16, tag="wv")
            wkr = consts.tile([128, 8, 128], BF16, tag="wk")
            wqr = consts.tile([128, 8, 128], BF16, tag="wq")
            nc.gpsimd.dma_start(wvr[:], wv_d.ap()[:])
            nc.sync.dma_start(wkr[:], wk_d.ap()[:])
            nc.scalar.dma_start(wqr[:], wq_d.ap()[:])
            wpr = consts.tile([128, D], BF16, tag="wp")
            nc.gpsimd.dma_start(wpr[:], wp_d.ap()[:])
            bq_sb = consts.tile([128, 1], dt.float32, tag="bq")
            bk_sb = consts.tile([128, 1], dt.float32, tag="bk")
            bv_sb = consts.tile([128, 1], dt.float32, tag="bv")
            nc.gpsimd.dma_start(bq_sb[:], bq_d.ap()[:])
            nc.gpsimd.dma_start(bk_sb[:], bk_d.ap()[:])
            nc.gpsimd.dma_start(bv_sb[:], bv_d.ap()[:])
            msku = consts.tile([128, 128], BF16, tag="msku")
            nc.gpsimd.dma_start(msku[:], msk_d.ap()[:])
            ident = consts.tile([128, 128], BF16, tag="ident")
            nc.gpsimd.dma_start(ident[:], id_d.ap()[:])
            ones_r = consts.tile([128, 64], BF16, tag="ones")
            nc.gpsimd.dma_start(ones_r[:], ones_d.ap()[:, :])

            # ---- x loads: full rows, both batches, upfront ----
            xd = [
                [
                    xpool.tile([128, S], BF16, tag="x", name=f"x_{b}_{d}")
                    for d in range(8)
                ]
                for b in range(B)
            ]
            for b in range(B):
                for d in range(8):
                    eng = nc.sync if d % 2 == 0 else nc.scalar
                    eng.dma_start(
                        xd[b][d][:],
                        xT_d.ap()[b, 128 * d:128 * (d + 1), :],
                    )

            # pending proj work items: (b, blk, dtile, aT) emitted one at a
            # time, spread through the next block's j loop so the PSUM ring
            # and the DVE y-copies never gate the PE.
            proj_q = []

            def emit_proj_one(copy_scalar=False):
                if not proj_q:
                    return
                b, blk, dtile, aT = proj_q.pop(0)
                si0 = 512 * blk
                ps = ps_aux.tile([128, 512], dt.float32, tag="aux",
                                 name=f"psp_{b}_{blk}_{dtile}")
                nc.tensor.matmul(
                    ps[:],
                    wpr[:, 128 * dtile:128 * (dtile + 1)],
                    aT[:, si0:si0 + 512],
                    start=True,
                    stop=True,
                )
                y_sb = ypool.tile([128, 512], BF16, tag="y",
                                  name=f"y_{b}_{blk}_{dtile}")
                with nc.allow_low_precision(reason="bf16 partial out"):
                    if copy_scalar:
                        nc.scalar.activation(y_sb[:], ps[:], AF.Copy)
                    else:
                        nc.vector.tensor_copy(y_sb[:], ps[:])
                dma_eng = nc.sync if dtile % 2 == 0 else nc.gpsimd
                dma_eng.dma_start(
                    yT_d.ap()[
                        b, 128 * dtile:128 * (dtile + 1), si0:si0 + 512,
                    ],
                    y_sb[:],
                )

            def queue_proj(b, blk, aT):
                for dtile in range(8):
                    proj_q.append((b, blk, dtile, aT))

            for b in range(B):
                qTr = qkvp.tile([128, S], BF16, tag="qT", name=f"qT_{b}")
                kp0 = qkvp.tile([128, S], BF16, tag="kp0", name=f"kp0_{b}")
                kp1 = qkvp.tile([128, S], BF16, tag="kp1", name=f"kp1_{b}")
                nc.gpsimd.dma_start(kp0[64:128, :], zer_d.ap()[:])
                nc.gpsimd.dma_start(kp1[0:64, :], zer_d.ap()[:])
                vT = vpool.tile([128, S], BF16, tag="vT", name=f"vT_{b}")

                for bp in range(2):
                    for (w_r, bias, kind) in (
                        (wvr, bv_sb, "v"),
                        (wkr, bk_sb, "k"),
                        (wqr, bq_sb, "q"),
                    ):
                        pp = ps_mm2.tile([128, 2, 512], dt.float32, tag="mm2",
                                         name=f"qkv_{b}_{kind}_{bp}")
                        for d in range(8):
                            for t in range(2):
                                blk = 2 * bp + t
                                nc.tensor.matmul(
                                    pp[:, t, :],
                                    w_r[:, d, :],
                                    xd[b][d][:, 512 * blk:512 * (blk + 1)],
                                    start=(d == 0),
                                    stop=(d == 7),
                                )
                        cols = slice(1024 * bp, 1024 * (bp + 1))
                        src_ap = pp[:].rearrange("p t f -> p (t f)")
                        with nc.allow_low_precision(reason="bf16 qkv bias"):
                            if kind == "q":
                                nc.vector.tensor_scalar_add(
                                    qTr[:, cols], src_ap, bias[:, 0:1])
                            elif kind == "v":
                                nc.vector.tensor_scalar_add(
                                    vT[:, cols], src_ap, bias[:, 0:1])
                            else:
                                nc.vector.tensor_scalar_add(
                                    kp0[0:64, cols], src_ap[0:64, :],
                                    bias[0:64, 0:1])
                                nc.vector.tensor_scalar_add(
                                    kp1[64:128, cols], src_ap[64:128, :],
                                    bias[64:128, 0:1])

                # vhat: v natural per sj tile + ones column, bf16.
                vhat = qkvp.tile([128, 16, 130], BF16, tag="vhat",
                                 name=f"vhat_{b}")
                nc.gpsimd.dma_start(vhat[:, :, 64], ones_d.ap()[:, 0:16])
                nc.gpsimd.dma_start(vhat[:, :, 129], ones_d.ap()[:, 16:32])

                def emit_vhat(j):
                    pst = ps_aux.tile([128, 128], BF16, tag="aux",
                                      name=f"tr_{b}_{j}")
                    nc.tensor.transpose(
                        pst[:], vT[:, 128 * j:128 * (j + 1)], ident[:]
                    )
                    with nc.allow_low_precision(reason="bf16 vhat"):
                        nc.vector.tensor_copy(vhat[:, j, 0:64], pst[:, 0:64])
                        nc.vector.tensor_copy(vhat[:, j, 65:129],
                                              pst[:, 64:128])

                aT = qkvp.tile([128, S], BF16, tag="aT", name=f"aT_{b}")

                # ---- causal attention, lag-2 pipelined over j ----
                for blk in range(4):
                    si0 = 512 * blk
                    jlast = 4 * blk + 3
                    for j in range(4 * blk, 4 * blk + 4):
                        emit_vhat(j)
                    psa = [
                        ps_a_pool.tile([65, 512], dt.float32, tag="acc",
                                       name=f"psa_{b}_{blk}_{hl}")
                        for hl in range(HPC)
                    ]
                    ees = {}

                    def emit_scores(j):
                        off = max(0, 128 * (j - 4 * blk))
                        w = 512 - off
                        pp = ps_mm2.tile([128, 2, 512], dt.float32, tag="mm2",
                                         name=f"pp_{b}_{blk}_{j}")
                        diag = j >= 4 * blk
                        for hl, kp in ((0, kp0), (1, kp1)):
                            if diag:
                                # causal mask: -240 strict-upper, first in
                                # the accumulation group
                                nc.tensor.matmul(
                                    pp[:, hl, 0:128],
                                    msku[:],
                                    ident[:],
                                    start=True,
                                    stop=False,
                                )
                            nc.tensor.matmul(
                                pp[:, hl, 0:w],
                                kp[:, 128 * j:128 * (j + 1)],
                                qTr[:, si0 + off:si0 + 512],
                                start=not diag,
                                stop=True,
                            )
                        ee = epool.tile([128, 2, 512], BF16, tag="eT",
                                        name=f"ee_{b}_{blk}_{j}")
                        nc.scalar.activation(
                            ee[:, :, 0:w], pp[:, :, 0:w], AF.Exp, scale=0.125
                        )
                        ees[j] = (ee, off, w)

                    def emit_pv(j):
                        ee, off, w = ees.pop(j)
                        for hl in range(HPC):
                            nc.tensor.matmul(
                                psa[hl][:, off:512],
                                vhat[:, j, 65 * hl:65 * hl + 65],
                                ee[:, hl, 0:w],
                                start=(j == 0),
                                stop=(j == jlast),
                            )

                    LAG = 2
                    nproj = 2 if jlast + 1 < 8 else 1
                    for j in range(jlast + 1):
                        emit_scores(j)
                        if j >= LAG:
                            emit_pv(j - LAG)
                        if j >= 1:
                            for _ in range(nproj):
                                emit_proj_one()
                    for j in range(max(0, jlast + 1 - LAG), jlast + 1):
                        emit_pv(j)

                    # start the denominator/reciprocal chain on scalar+DVE
                    # immediately; the PE does proj(blk-1) meanwhile, so the
                    # psb broadcast matmul never waits.
                    recs = []
                    for hl in range(HPC):
                        den = rpool.tile([1, 512], dt.float32, tag="den",
                                         name=f"den_{b}_{blk}_{hl}")
                        nc.scalar.activation(den[:], psa[hl][64:65, :],
                                             AF.Copy)
                        rec_f = rpool.tile([1, 512], dt.float32, tag="rec_f",
                                           name=f"recf_{b}_{blk}_{hl}")
                        nc.vector.reciprocal_approx_fast(rec_f[:], den[:])
                        rec = rpool.tile([1, 512], BF16, tag="rec",
                                         name=f"rec_{b}_{blk}_{hl}")
                        with nc.allow_low_precision(reason="bf16 recip"):
                            nc.vector.tensor_copy(rec[:], rec_f[:])
                        recs.append(rec)
                    while proj_q:
                        emit_proj_one()
                    for hl in range(HPC):
                        p0 = 64 * hl
                        psb = ps_aux.tile([64, 512], dt.float32, tag="aux",
                                          name=f"psb_{b}_{blk}_{hl}")
                        nc.tensor.matmul(
                            psb[:], ones_r[0:1, 0:64], recs[hl][:],
                            start=True, stop=True
                        )
                        a_sb = rpool.tile([64, 512], BF16, tag="a_sb",
                                          name=f"asb_{b}_{blk}_{hl}")
                        with nc.allow_low_precision(reason="bf16 attn norm"):
                            nc.vector.tensor_copy(a_sb[:], psa[hl][0:64, :])
                            nc.vector.tensor_mul(
                                aT[p0:p0 + 64, si0:si0 + 512],
                                a_sb[:],
                                psb[:],
                            )
                    queue_proj(b, blk, aT)
            k = 0
            while proj_q:
                emit_proj_one(copy_scalar=(k % 2 == 1))
                k += 1
    nc.compile()
    return nc


def _get_nc():
    if "nc" not in _CACHE:
        _CACHE["nc"] = build_nc()
    return _CACHE["nc"]


def prep_w(w):
    # [1024, 128] -> [128(p), 8(d), 128(m)] so the SBUF load is contiguous
    return np.ascontiguousarray(
        w.reshape(8, 128, 128).transpose(1, 0, 2)
    ).astype(ml_dtypes.bfloat16)


def make_in_maps(x, W_attn, b_attn, W_proj):
    x = np.ascontiguousarray(x, dtype=np.float32)
    xT = np.ascontiguousarray(x.transpose(0, 2, 1)).astype(ml_dtypes.bfloat16)

    p = np.arange(128)
    # strict upper-triangular -240: out[sj, si] += msku[si, sj] via
    # matmul(lhsT=msku, rhs=ident) -> -240 where sj > si
    msku = np.where(p[None, :] > p[:, None], -240.0, 0.0)
    msku = msku.astype(ml_dtypes.bfloat16)
    ident = np.eye(128, dtype=np.float32).astype(ml_dtypes.bfloat16)
    ones = np.ones((128, 64), ml_dtypes.bfloat16)

    in_maps = []
    for c in range(NCORE):
        col0 = HD * HPC * c
        in_maps.append({
            "xT": xT,
            "wq": prep_w(W_attn[:, col0:col0 + 128]),
            "wk": prep_w(W_attn[:, D + col0:D + col0 + 128]),
            "wv": prep_w(W_attn[:, 2 * D + col0:2 * D + col0 + 128]),
            "bq": np.ascontiguousarray(b_attn[col0:col0 + 128].reshape(128, 1)).astype(np.float32),
            "bk": np.ascontiguousarray(b_attn[D + col0:D + col0 + 128].reshape(128, 1)).astype(np.float32),
            "bv": np.ascontiguousarray(b_attn[2 * D + col0:2 * D + col0 + 128].reshape(128, 1)).astype(np.float32),
            "wp": np.ascontiguousarray(W_proj[128 * c:128 * (c + 1), :]).astype(ml_dtypes.bfloat16),
            "msku": msku,
            "zer": np.zeros((64, S), ml_dtypes.bfloat16),
            "ident": ident,
            "ones": ones,
        })
    return in_maps


def gather(results, b_proj):
    acc = np.zeros((B, D, S), np.float32)
    for r in results:
        acc += np.asarray(r["yT"], dtype=np.float32)
    out = acc.transpose(0, 2, 1) + np.asarray(b_proj, np.float32)[None, None, :]
    return np.ascontiguousarray(out.astype(np.float32))


def kernel(x, W_attn, b_attn, W_proj, b_proj, _trace=False, _trace_kwargs=None):
    nc = _get_nc()
    in_maps = make_in_maps(np.asarray(x), np.asarray(W_attn),
                           np.asarray(b_attn), np.asarray(W_proj))
    res = run_bass_kernel_spmd(
        nc, in_maps, list(range(NCORE)), trace=_trace, **(_trace_kwargs or {})
    )
    out = gather(res.results, np.asarray(b_proj))
    if _trace:
        kernel.last_result = res
    return out
